# revision 39
# baseline (speedup 1.0000x reference)
"""Masked-attention kernel for 8 TRN2 NeuronCores (batch-parallel sharding).

Layout strategy: all transposes/dtype prep are done on the HOST (numpy)
so the device touches data only in matmul-native layouts:
  - Q, K are pre-transposed on host to [B, D, S] fp16; DMA lands them
    directly as [d=partition, s] tiles (contiguous per-partition runs).
    No PE transposes, no staging copies, no on-device casts.
  - scores are computed transposed (S^T[k, q]) so the PV matmul consumes
    exp() output directly with V in its natural [k, d] layout; per k-tile
    one QK matmul pair (N=512 fp16) accumulates into a PSUM score tile,
    exp runs on ACT over [128, 1024] with the 1/sqrt(dk) scale folded in.
  - masking: the host pre-transposes and inverts the mask into fp16
    keep-multiplicands [B, S_k, S_q] (1.0 = keep); after exp, the DVE
    multiplies each pt tile by its keep tile in place (masked lanes -> 0
    in both the PV numerator and the denominator). No PE mask matmuls,
    no -inf adds: exp of unmasked scores is bounded (~e^6), so fp16 is
    safe. PV lags 3 k-tiles so the exp->mult chain stays off the PE
    critical path.
  - exp outputs land in [128, 2, 1024] pair tiles; the softmax-denominator
    accumulation is one FD=2048 DVE add per k-tile pair; one all-ones
    [128,128] stationary matmul broadcasts den[q] to every PSUM partition;
    DVE reciprocal + one multiply normalize the accumulated PV output.
  - output is stored transposed [B, D, S] and un-transposed on host.
  - DMA ring assignment: keep-mask stream on the SP HWDGE ring, Q/K/V
    prep loads on the ACT HWDGE ring (so the exp stream never queues
    behind a parked DMA wait), output stores on SWDGE (idle Pool engine).
"""

import numpy as np
import ml_dtypes

B, S, D = 16, 2048, 128
NCORES = 8
BP = B // NCORES  # batches per core
P = 128
QC = 1024  # q-chunk (columns of the transposed score tile)
NQC = S // QC
NKT = S // P  # k tiles
MM_N = 512  # matmul moving free dim (one PSUM bank of fp32)
SCALE = 1.0 / float(np.sqrt(128.0))
MASK_NEG = -240.0

_CACHE = {}

# dev-only ablation switches (see ablate.py); empty for the graded path
ABLATE = {}

# masking scheme: True = all k-tiles masked post-exp on the DVE (no PE
# mask matmuls); False = parity split PE/DVE
ALL_DVE = True


def build_nc(loop=True):
    import concourse.mybir as mybir
    import concourse.tile as tile
    from concourse import bacc

    fp16 = mybir.dt.float16
    fp32 = mybir.dt.float32

    nc = bacc.Bacc("TRN2", target_bir_lowering=False, debug=False,
                   num_devices=NCORES)

    Qtd = nc.dram_tensor("Qt", [BP, D, S], fp16, kind="ExternalInput")
    Ktd = nc.dram_tensor("Kt", [BP, D, S], fp16, kind="ExternalInput")
    Vd = nc.dram_tensor("V", [BP, S, D], fp16, kind="ExternalInput")
    if ALL_DVE:
        Md = None
        Kd = nc.dram_tensor("keepT", [BP, S, S], fp16,
                            kind="ExternalInput")
    else:
        Md = nc.dram_tensor("maskT", [BP, S // 2, S], mybir.dt.float8e4,
                            kind="ExternalInput")
        Kd = nc.dram_tensor("keepT", [BP, S // 2, S], fp16,
                            kind="ExternalInput")
    if loop:
        # run-count knob for differential HW timing (graded path: loop=False)
        Id = nc.dram_tensor("iters", [1, 1], mybir.dt.int32,
                            kind="ExternalInput")
    Od = nc.dram_tensor("outT", [BP, D, S], fp32, kind="ExternalOutput")

    negI_np = (MASK_NEG * np.eye(P, dtype=np.float32)).astype(
        ml_dtypes.float8_e4m3)
    negI_dram = nc.inline_tensor(negI_np, name="negI_const")
    ones_dram = nc.inline_tensor(np.ones((P, P), dtype=np.float16),
                                 name="ones_const")

    with tile.TileContext(nc) as tc:
        with tc.tile_pool(name="consts", bufs=1) as consts, \
             tc.tile_pool(name="stag", bufs=2) as stag, \
             tc.tile_pool(name="qkv", bufs=1) as qkv, \
             tc.tile_pool(name="maskp",
                          bufs=(3 if ABLATE.get("mkt8") else 5)) as maskp, \
             tc.tile_pool(name="pp",
                          bufs=(4 if ABLATE.get("v6") else 6)) as pp, \
             tc.tile_pool(name="accp",
                          bufs=(2 if ABLATE.get("v6") else 3)) as accp, \
             tc.tile_pool(name="outp", bufs=2) as outp, \
             tc.tile_pool(name="spsum", bufs=3, space="PSUM") as spsum, \
             tc.tile_pool(name="opsum", bufs=1, space="PSUM") as opsum:

            negI = consts.tile([P, P], mybir.dt.float8e4)
            nc.sync.dma_start(out=negI[:, :], in_=negI_dram.ap())
            ones_mat = consts.tile([P, P], fp16)
            nc.sync.dma_start(out=ones_mat[:, :], in_=ones_dram.ap())

            pools = (stag, qkv, maskp, pp, accp, outp, spsum, opsum)
            if loop:
                it_sb = consts.tile([1, 1], mybir.dt.int32)
                nc.sync.dma_start(out=it_sb[:, :], in_=Id.ap())
                n_iters = nc.values_load(it_sb[:, :],
                                         skip_runtime_bounds_check=True)
                with tc.For_i(0, n_iters, 1,
                              hint_engines=(mybir.EngineType.PE,
                                            mybir.EngineType.Activation,
                                            mybir.EngineType.DVE,
                                            mybir.EngineType.SP,
                                            mybir.EngineType.Pool)):
                    _kernel_body(nc, mybir, Qtd, Ktd, Vd, Md, Kd, Od,
                                 negI, ones_mat, *pools)
            else:
                _kernel_body(nc, mybir, Qtd, Ktd, Vd, Md, Kd, Od,
                             negI, ones_mat, *pools)
    nc.compile()
    return nc


def _kernel_body(nc, mybir, Qtd, Ktd, Vd, Md, Kd, Od, negI, ones_mat,
                 stag, qkv, maskp, pp, accp, outp, spsum, opsum):
    fp16 = mybir.dt.float16
    fp32 = mybir.dt.float32
    fp8 = mybir.dt.float8e4
    Exp = mybir.ActivationFunctionType.Exp

    HS = S // 2  # half of the s dimension, for chunked loads
    AB = ABLATE
    MKT = 8 if AB.get("mkt8") else 4  # k-tiles per keep DMA

    def load_mask_pair(b, qc, mt):
        # PE-mask tiles: even k-tiles, [k=partition, group, q] fp8 (host
        # pre-encoded; plain byte copy on the SP HWDGE ring)
        t = maskp.tile([P, MKT, QC], fp8, name="mfT")
        nc.sync.dma_start(
            out=t[:, :, :],
            in_=Md.ap()[b, mt * MKT * P:(mt + 1) * MKT * P,
                        qc * QC:(qc + 1) * QC]
                .rearrange("(t p) q -> p t q", p=P))
        return t

    def load_keep_pair(b, qc, mt):
        # DVE-keep tiles: odd k-tiles, fp16 1.0/0.0 multiplicands
        t = maskp.tile([P, MKT, QC], fp16, name="kfT")
        ring = nc.gpsimd.dma_start if AB.get("keep_swdge") \
            else nc.sync.dma_start
        ring(
            out=t[:, :, :],
            in_=Kd.ap()[b, mt * MKT * P:(mt + 1) * MKT * P,
                        qc * QC:(qc + 1) * QC]
                .rearrange("(t p) q -> p t q", p=P))
        return t

    def prep_batch(b, mf0):
        # Q^T/K^T land directly as [d, s] fp16 (host pre-transposed and
        # pre-cast); V natural fp16. Halves on the two HWDGE rings.
        ktt = qkv.tile([P, S], fp16, name=f"ktt{b}")
        qt = qkv.tile([P, S], fp16, name=f"qt{b}")
        vsb = qkv.tile([P, NKT, D], fp16, name=f"vsb{b}")

        QS = S // 4

        def half(dst, src_ap, h, ring, vshape=False):
            if vshape:
                ring(out=dst[:, h * (NKT // 2):(h + 1) * (NKT // 2), :],
                     in_=src_ap[b, h * HS:(h + 1) * HS, :]
                         .rearrange("(t p) d -> p t d", p=P))
            else:
                # quarter-granular writer DMAs: the first QK matmul only
                # waits on the first 512 columns, not the whole half
                for g in (2 * h, 2 * h + 1):
                    ring(out=dst[:, g * QS:(g + 1) * QS],
                         in_=src_ap[b, :, g * QS:(g + 1) * QS])

        # prep loads ride the ACT ring: it is otherwise empty, their
        # waits are satisfied by the time they issue, and the exp stream
        # never sits behind a parked DMA wait
        ld = nc.sync.dma_start if AB.get("rings") == "v4" \
            else nc.scalar.dma_start
        half(ktt, Ktd.ap(), 0, ld)
        half(qt, Qtd.ap(), 0, nc.scalar.dma_start)
        if mf0 is not None:
            if not ALL_DVE:
                mf0.append(load_mask_pair(b, 0, 0))
            mf0.append(load_keep_pair(b, 0, 0))
        half(vsb, Vd.ap(), 0, ld, vshape=True)
        half(ktt, Ktd.ap(), 1, ld)
        half(qt, Qtd.ap(), 1, nc.scalar.dma_start)
        half(vsb, Vd.ap(), 1, ld, vshape=True)
        return qt, ktt, vsb

    const_pt = None
    if AB.get("pv_const_pt"):
        const_pt = qkv.tile([P, 2, QC], fp16, name="constpt")
        nc.vector.memset(const_pt, 0.001)

    mf00 = []
    prepped = {0: prep_batch(0, mf00)}

    # ---- main flash loop over (batch, q-chunk, k-tile) ----
    for b in range(BP):
        for qc in range(NQC):
            if (b, qc) == (0, 1) and BP > 1:
                prepped[1] = prep_batch(1, None)
            qt, ktt, vsb = prepped[b]
            NKEEP = NKT if ALL_DVE else NKT // 2
            NEG = NKEEP // MKT  # keep (and mask) DMA groups per q-chunk
            if AB.get("no_mask_dma"):
                mf = [mf00[0]] * NEG
                kf = [mf00[-1]] * NEG
            elif b == 0 and qc == 0:
                mf = [mf00[0]] + ([] if ALL_DVE else
                                  [load_mask_pair(b, qc, mt)
                                   for mt in range(1, NEG)])
                kf = [mf00[-1]] + [load_keep_pair(b, qc, mt)
                                   for mt in range(1, NEG)]
            else:
                mf = ([] if ALL_DVE else
                      [load_mask_pair(b, qc, mt) for mt in range(NEG)])
                kf = [load_keep_pair(b, qc, mt) for mt in range(NEG)]
            acc = accp.tile([P, 2, QC], fp16, name="acc")
            if AB.get("no_acc") or AB.get("no_exp"):
                nc.vector.memset(acc, 1.0)
            ops = opsum.tile([P, QC], fp32, name="opsum")
            pts = {}

            swap0 = 0 if AB.get("parity_even") else 1

            def pv_lag(j):
                # DVE-masked tiles have the keep-mult in their chain: one
                # extra k-tile of lag keeps it off the PV critical path
                if AB.get("v6"):
                    return 2
                if ALL_DVE:
                    return 4 if AB.get("lag4") else 3
                return 2 if j % 2 == swap0 else 3
            for kt in range(NKT):
                sc = spsum.tile([P, QC], fp32, name="scores")
                # parity split: even k-tiles fold the mask on the PE (fp8
                # matmul into the score accumulation); odd k-tiles apply it
                # post-exp as a DVE multiply by the fp16 keep tile
                swap = 0 if AB.get("parity_even") else 1
                pe_mask = (not ALL_DVE) and (kt % 2 == swap) \
                    and not AB.get("no_mask_mm")
                i = kt if ALL_DVE else kt // 2
                if pe_mask:
                    mfck = mf[i // MKT]
                    for n in range(0, QC, MM_N):
                        # start=True clears the 512-wide PSUM bank; the mask
                        # matmul leads each bank's accumulation group
                        nc.tensor.matmul(
                            sc[:, n:n + MM_N],
                            lhsT=negI[:, :],
                            rhs=mfck[:, i % MKT, n:n + MM_N],
                            start=True, stop=AB.get("no_qk", False),
                            skip_group_check=True)
                if not AB.get("no_qk"):
                    for n in range(0, QC, MM_N):
                        nc.tensor.matmul(
                            sc[:, n:n + MM_N],
                            lhsT=ktt[:, kt * P:(kt + 1) * P],
                            rhs=qt[:, qc * QC + n:qc * QC + n + MM_N],
                            start=not pe_mask, stop=True,
                            skip_group_check=True)
                # exp outputs land in [P, 2, QC] pair tiles so the acc
                # update is one FD=2048 add per k-tile pair (half the DVE
                # instructions and semaphores on the acc chain)
                if kt % 2 == 0:
                    ptp = pp.tile([P, 2, QC], fp16, name="ptpair")
                    slot = 0
                else:
                    slot = 1
                pt = ptp[:, slot, :]
                if not AB.get("no_exp"):
                    nc.scalar.activation(out=pt, in_=sc[:, :],
                                         func=Exp, scale=SCALE)
                elif const_pt is None:
                    nc.vector.tensor_copy(out=pt, in_=sc[:, :])
                do_mult = (ALL_DVE or kt % 2 != swap) \
                    and not AB.get("no_mask_mm") \
                    and not AB.get("no_mult") and not AB.get("no_exp")
                if do_mult and ALL_DVE and kt % 2 == 1:
                    # one in-place FD=2048 keep-multiply per k-tile pair
                    kbase = (kt - 1) % MKT
                    nc.vector.tensor_mul(
                        out=ptp[:, :, :], in0=ptp[:, :, :],
                        in1=kf[(kt - 1) // MKT][:, kbase:kbase + 2, :])
                elif do_mult and not ALL_DVE:
                    # in-place keep-multiply on the DVE-masked slot
                    nc.vector.tensor_mul(out=pt, in0=pt,
                                         in1=kf[i // MKT][:, i % MKT, :])
                if AB.get("no_acc") or AB.get("no_exp") or kt % 2 == 0:
                    pass
                elif kt == 1:
                    nc.vector.tensor_copy(out=acc[:, :, :],
                                          in_=ptp[:, :, :])
                else:
                    nc.vector.tensor_add(out=acc[:, :, :],
                                         in0=acc[:, :, :],
                                         in1=ptp[:, :, :])
                if const_pt is not None:
                    pts[kt] = (const_pt, 0)
                else:
                    pts[kt] = (ptp, slot)
                if not AB.get("no_pv"):
                    # ascending j so the start=True (j==0) matmul is always
                    # the first write into the ops bank
                    for j in (kt - 3, kt - 2):
                        if j < 0 or j + pv_lag(j) != kt:
                            continue
                        tj, sl = pts[j]
                        for n in range(0, QC, MM_N):
                            nc.tensor.matmul(
                                ops[:, n:n + MM_N],
                                lhsT=vsb[:, j, :],
                                rhs=tj[:, sl, n:n + MM_N],
                                start=(j == 0), stop=False,
                                skip_group_check=True)
            if not AB.get("no_pv"):
                for j in range(NKT):
                    if j + pv_lag(j) < NKT:
                        continue
                    tj, sl = pts[j]
                    for n in range(0, QC, MM_N):
                        nc.tensor.matmul(
                            ops[:, n:n + MM_N],
                            lhsT=vsb[:, j, :],
                            rhs=tj[:, sl, n:n + MM_N],
                            start=False, stop=(j == NKT - 1),
                            skip_group_check=True)

            # denominator broadcast to all partitions in one matmul:
            # den[p, q] = sum_k acc[k, q] for every p (all-ones stationary)
            den = spsum.tile([P, QC], fp32, name="scores")
            for n in range(0, QC, MM_N):
                for sl in range(2):
                    nc.tensor.matmul(den[:, n:n + MM_N],
                                     lhsT=ones_mat[:, :],
                                     rhs=acc[:, sl, n:n + MM_N],
                                     start=(sl == 0), stop=(sl == 1),
                                     skip_group_check=True)
            rcp = outp.tile([P, QC], fp32, name="rcp")
            nc.vector.reciprocal(out=rcp[:, :], in_=den[:, :])
            osf = outp.tile([P, QC], fp32, name="osf")
            nc.vector.tensor_mul(out=osf[:, :], in0=ops[:, :],
                                 in1=rcp[:, :])
            # stores go out on SWDGE: their wait (epilogue mult) parks on
            # the idle Pool engine instead of a ring the PE feed needs
            if AB.get("rings") == "v4":
                st = nc.scalar.dma_start if qc % 2 == 0 \
                    else nc.sync.dma_start
            else:
                st = nc.gpsimd.dma_start
            st(out=Od.ap()[b, :, qc * QC:(qc + 1) * QC], in_=osf[:, :])


def _get_nc(loop=False):
    key = f"nc_loop{loop}"
    if key not in _CACHE:
        _CACHE[key] = build_nc(loop=loop)
    return _CACHE[key]


def host_prep(Q, K, V, mask):
    """Slice per core, pre-transpose Q/K/mask, pre-cast to device dtypes.

    The transposed mask is split by k-tile parity: even k-tiles are packed
    as raw fp8e4m3 bytes (0x38 = 1.0) for the PE mask-matmul path; odd
    k-tiles are packed as fp16 keep-multiplicands (1.0 = keep) for the
    DVE post-exp multiply."""
    Q = np.asarray(Q, dtype=np.float32)
    K = np.asarray(K, dtype=np.float32)
    V16 = np.asarray(V, dtype=np.float32).astype(np.float16)
    mask_u8 = np.asarray(mask).astype(np.uint8)
    Qt = np.ascontiguousarray(
        Q.transpose(0, 2, 1)).astype(np.float16)
    Kt = np.ascontiguousarray(
        K.transpose(0, 2, 1)).astype(np.float16)
    mT = mask_u8.transpose(0, 2, 1).reshape(B, NKT, P, S)
    if ALL_DVE:
        maskT = None
        keepT = np.ascontiguousarray(
            (1 - mT).astype(np.float16)).reshape(B, S, S)
    else:
        maskT = np.ascontiguousarray(
            mT[:, 1::2] * np.uint8(0x38)).reshape(B, S // 2, S)
        maskT = maskT.view(ml_dtypes.float8_e4m3)
        keepT = np.ascontiguousarray(
            (1 - mT[:, 0::2]).astype(np.float16)).reshape(B, S // 2, S)
    in_maps = []
    for c in range(NCORES):
        sl = slice(c * BP, (c + 1) * BP)
        m = {
            "Qt": np.ascontiguousarray(Qt[sl]),
            "Kt": np.ascontiguousarray(Kt[sl]),
            "V": np.ascontiguousarray(V16[sl]),
            "keepT": np.ascontiguousarray(keepT[sl]),
        }
        if maskT is not None:
            m["maskT"] = np.ascontiguousarray(maskT[sl])
        in_maps.append(m)
    return in_maps


def gather_out(results):
    """Concat per-core transposed outputs and un-transpose to [B, S, D]."""
    outT = np.concatenate([r["outT"] for r in results], axis=0)
    return np.ascontiguousarray(outT.transpose(0, 2, 1))


def kernel(Q, K, V, mask, dk=128):
    from concourse.bass_utils import run_bass_kernel_spmd

    assert int(dk) == 128
    in_maps = host_prep(Q, K, V, mask)
    nc = _get_nc(loop=False)
    res = run_bass_kernel_spmd(nc, in_maps, core_ids=list(range(NCORES)))
    return gather_out(res.results)


# revision 42
# speedup vs baseline: 1.0088x; 1.0088x over previous
"""Masked-attention kernel for 8 TRN2 NeuronCores (batch-parallel sharding).

Layout strategy: all transposes/dtype prep are done on the HOST (numpy)
so the device touches data only in matmul-native layouts:
  - Q, K are pre-transposed on host to [B, D, S] fp16; DMA lands them
    directly as [d=partition, s] tiles (contiguous per-partition runs).
    No PE transposes, no staging copies, no on-device casts.
  - scores are computed transposed (S^T[k, q]) so the PV matmul consumes
    exp() output directly with V in its natural [k, d] layout; per k-tile
    one QK matmul pair (N=512 fp16) accumulates into a PSUM score tile,
    exp runs on ACT over [128, 1024] with the 1/sqrt(dk) scale folded in.
  - masking: the host pre-transposes and inverts the mask into fp16
    keep-multiplicands [B, S_k, S_q] (1.0 = keep); after exp, the DVE
    multiplies each pt tile by its keep tile in place (masked lanes -> 0
    in both the PV numerator and the denominator). No PE mask matmuls,
    no -inf adds: exp of unmasked scores is bounded (~e^6), so fp16 is
    safe. PV lags 3 k-tiles so the exp->mult chain stays off the PE
    critical path.
  - exp outputs land in [128, 2, 1024] pair tiles; the softmax-denominator
    accumulation is one FD=2048 DVE add per k-tile pair; one all-ones
    [128,128] stationary matmul broadcasts den[q] to every PSUM partition;
    DVE reciprocal + one multiply normalize the accumulated PV output.
  - output is stored transposed [B, D, S] and un-transposed on host.
  - DMA ring assignment: keep-mask stream on the SP HWDGE ring, Q/K/V
    prep loads on the ACT HWDGE ring (so the exp stream never queues
    behind a parked DMA wait), output stores on SWDGE (idle Pool engine).
"""

import numpy as np
import ml_dtypes

B, S, D = 16, 2048, 128
NCORES = 8
BP = B // NCORES  # batches per core
P = 128
QC = 1024  # q-chunk (columns of the transposed score tile)
NQC = S // QC
NKT = S // P  # k tiles
MM_N = 512  # matmul moving free dim (one PSUM bank of fp32)
SCALE = 1.0 / float(np.sqrt(128.0))
MASK_NEG = -240.0

_CACHE = {}

# dev-only ablation switches (see ablate.py); empty for the graded path
ABLATE = {}

# masking scheme: True = all k-tiles masked post-exp on the DVE (no PE
# mask matmuls); False = parity split PE/DVE
ALL_DVE = True


def build_nc(loop=True):
    import concourse.mybir as mybir
    import concourse.tile as tile
    from concourse import bacc

    fp16 = mybir.dt.float16
    fp32 = mybir.dt.float32

    nc = bacc.Bacc("TRN2", target_bir_lowering=False, debug=False,
                   num_devices=NCORES)

    Qtd = nc.dram_tensor("Qt", [BP, D, S], fp16, kind="ExternalInput")
    Ktd = nc.dram_tensor("Kt", [BP, D, S], fp16, kind="ExternalInput")
    Vd = nc.dram_tensor("V", [BP, S, D], fp16, kind="ExternalInput")
    if ALL_DVE:
        Md = None
        Kd = nc.dram_tensor("keepT", [BP, S, S], fp16,
                            kind="ExternalInput")
    else:
        Md = nc.dram_tensor("maskT", [BP, S // 2, S], mybir.dt.float8e4,
                            kind="ExternalInput")
        Kd = nc.dram_tensor("keepT", [BP, S // 2, S], fp16,
                            kind="ExternalInput")
    if loop:
        # run-count knob for differential HW timing (graded path: loop=False)
        Id = nc.dram_tensor("iters", [1, 1], mybir.dt.int32,
                            kind="ExternalInput")
    Od = nc.dram_tensor("outT", [BP, D, S], fp32, kind="ExternalOutput")

    negI_np = (MASK_NEG * np.eye(P, dtype=np.float32)).astype(
        ml_dtypes.float8_e4m3)
    negI_dram = nc.inline_tensor(negI_np, name="negI_const")
    ones_dram = nc.inline_tensor(np.ones((P, P), dtype=np.float16),
                                 name="ones_const")

    with tile.TileContext(nc) as tc:
        with tc.tile_pool(name="consts", bufs=1) as consts, \
             tc.tile_pool(name="stag", bufs=2) as stag, \
             tc.tile_pool(name="qkv", bufs=1) as qkv, \
             tc.tile_pool(name="maskp",
                          bufs=(3 if ABLATE.get("mkt8") else 5)) as maskp, \
             tc.tile_pool(name="pp",
                          bufs=(4 if ABLATE.get("v6") else 6)) as pp, \
             tc.tile_pool(name="accp",
                          bufs=(2 if ABLATE.get("v6") else 3)) as accp, \
             tc.tile_pool(name="outp", bufs=2) as outp, \
             tc.tile_pool(name="spsum", bufs=3, space="PSUM") as spsum, \
             tc.tile_pool(name="opsum", bufs=1, space="PSUM") as opsum:

            negI = consts.tile([P, P], mybir.dt.float8e4)
            nc.sync.dma_start(out=negI[:, :], in_=negI_dram.ap())
            ones_mat = consts.tile([P, P], fp16)
            nc.sync.dma_start(out=ones_mat[:, :], in_=ones_dram.ap())

            pools = (stag, qkv, maskp, pp, accp, outp, spsum, opsum)
            if loop:
                it_sb = consts.tile([1, 1], mybir.dt.int32)
                nc.sync.dma_start(out=it_sb[:, :], in_=Id.ap())
                n_iters = nc.values_load(it_sb[:, :],
                                         skip_runtime_bounds_check=True)
                with tc.For_i(0, n_iters, 1,
                              hint_engines=(mybir.EngineType.PE,
                                            mybir.EngineType.Activation,
                                            mybir.EngineType.DVE,
                                            mybir.EngineType.SP,
                                            mybir.EngineType.Pool)):
                    _kernel_body(nc, mybir, Qtd, Ktd, Vd, Md, Kd, Od,
                                 negI, ones_mat, *pools)
            else:
                _kernel_body(nc, mybir, Qtd, Ktd, Vd, Md, Kd, Od,
                             negI, ones_mat, *pools)
    nc.compile()
    return nc


def _kernel_body(nc, mybir, Qtd, Ktd, Vd, Md, Kd, Od, negI, ones_mat,
                 stag, qkv, maskp, pp, accp, outp, spsum, opsum):
    fp16 = mybir.dt.float16
    fp32 = mybir.dt.float32
    fp8 = mybir.dt.float8e4
    Exp = mybir.ActivationFunctionType.Exp

    HS = S // 2  # half of the s dimension, for chunked loads
    AB = ABLATE
    MKT = 8 if AB.get("mkt8") else 4  # k-tiles per keep DMA

    def load_mask_pair(b, qc, mt):
        # PE-mask tiles: even k-tiles, [k=partition, group, q] fp8 (host
        # pre-encoded; plain byte copy on the SP HWDGE ring)
        t = maskp.tile([P, MKT, QC], fp8, name="mfT")
        nc.sync.dma_start(
            out=t[:, :, :],
            in_=Md.ap()[b, mt * MKT * P:(mt + 1) * MKT * P,
                        qc * QC:(qc + 1) * QC]
                .rearrange("(t p) q -> p t q", p=P))
        return t

    def load_keep_pair(b, qc, mt):
        # DVE-keep tiles: odd k-tiles, fp16 1.0/0.0 multiplicands
        t = maskp.tile([P, MKT, QC], fp16, name="kfT")
        ring = nc.gpsimd.dma_start if AB.get("keep_swdge") \
            else nc.sync.dma_start
        ring(
            out=t[:, :, :],
            in_=Kd.ap()[b, mt * MKT * P:(mt + 1) * MKT * P,
                        qc * QC:(qc + 1) * QC]
                .rearrange("(t p) q -> p t q", p=P))
        return t

    def prep_batch(b, mf0):
        # Q^T/K^T land directly as [d, s] fp16 (host pre-transposed and
        # pre-cast); V natural fp16. Halves on the two HWDGE rings.
        ktt = qkv.tile([P, S], fp16, name=f"ktt{b}")
        qt = qkv.tile([P, S], fp16, name=f"qt{b}")
        vsb = qkv.tile([P, NKT, D], fp16, name=f"vsb{b}")

        QS = S // 4

        def half(dst, src_ap, h, ring, vshape=False):
            if vshape:
                ring(out=dst[:, h * (NKT // 2):(h + 1) * (NKT // 2), :],
                     in_=src_ap[b, h * HS:(h + 1) * HS, :]
                         .rearrange("(t p) d -> p t d", p=P))
            else:
                # quarter-granular writer DMAs: the first QK matmul only
                # waits on the first 512 columns, not the whole half
                for g in (2 * h, 2 * h + 1):
                    ring(out=dst[:, g * QS:(g + 1) * QS],
                         in_=src_ap[b, :, g * QS:(g + 1) * QS])

        # prep loads ride the ACT ring: it is otherwise empty, their
        # waits are satisfied by the time they issue, and the exp stream
        # never sits behind a parked DMA wait
        ld = nc.sync.dma_start if AB.get("rings") == "v4" \
            else nc.scalar.dma_start
        half(ktt, Ktd.ap(), 0, ld)
        half(qt, Qtd.ap(), 0, nc.scalar.dma_start)
        if mf0 is not None:
            if not ALL_DVE:
                mf0.append(load_mask_pair(b, 0, 0))
            mf0.append(load_keep_pair(b, 0, 0))
        half(vsb, Vd.ap(), 0, ld, vshape=True)
        half(ktt, Ktd.ap(), 1, ld)
        half(qt, Qtd.ap(), 1, nc.scalar.dma_start)
        half(vsb, Vd.ap(), 1, ld, vshape=True)
        return qt, ktt, vsb

    const_pt = None
    if AB.get("pv_const_pt"):
        const_pt = qkv.tile([P, 2, QC], fp16, name="constpt")
        nc.vector.memset(const_pt, 0.001)

    mf00 = []
    prepped = {0: prep_batch(0, mf00)}

    # ---- main flash loop over (batch, q-chunk, k-tile) ----
    for b in range(BP):
        for qc in range(NQC):
            if (b, qc) == (0, 1) and BP > 1:
                prepped[1] = prep_batch(1, None)
            qt, ktt, vsb = prepped[b]
            NKEEP = NKT if ALL_DVE else NKT // 2
            NEG = NKEEP // MKT  # keep (and mask) DMA groups per q-chunk
            if AB.get("no_mask_dma"):
                mf = [mf00[0]] * NEG
                kf = [mf00[-1]] * NEG
            elif b == 0 and qc == 0:
                mf = [mf00[0]] + ([] if ALL_DVE else
                                  [load_mask_pair(b, qc, mt)
                                   for mt in range(1, NEG)])
                kf = [mf00[-1]] + [load_keep_pair(b, qc, mt)
                                   for mt in range(1, NEG)]
            else:
                mf = ([] if ALL_DVE else
                      [load_mask_pair(b, qc, mt) for mt in range(NEG)])
                kf = [load_keep_pair(b, qc, mt) for mt in range(NEG)]
            acc = accp.tile([P, 2, QC], fp16, name="acc")
            if AB.get("no_acc") or AB.get("no_exp"):
                nc.vector.memset(acc, 1.0)
            ops = opsum.tile([P, QC], fp32, name="opsum")
            pts = {}

            swap0 = 0 if AB.get("parity_even") else 1

            def pv_lag(j):
                # DVE-masked tiles have the keep-mult in their chain: one
                # extra k-tile of lag keeps it off the PV critical path
                if AB.get("v6"):
                    return 2
                if ALL_DVE:
                    return 4 if AB.get("lag4") else 3
                return 2 if j % 2 == swap0 else 3
            for kt in range(NKT):
                sc = spsum.tile([P, QC], fp32, name="scores")
                # parity split: even k-tiles fold the mask on the PE (fp8
                # matmul into the score accumulation); odd k-tiles apply it
                # post-exp as a DVE multiply by the fp16 keep tile
                swap = 0 if AB.get("parity_even") else 1
                pe_mask = (not ALL_DVE) and (kt % 2 == swap) \
                    and not AB.get("no_mask_mm")
                i = kt if ALL_DVE else kt // 2
                if pe_mask:
                    mfck = mf[i // MKT]
                    for n in range(0, QC, MM_N):
                        # start=True clears the 512-wide PSUM bank; the mask
                        # matmul leads each bank's accumulation group
                        nc.tensor.matmul(
                            sc[:, n:n + MM_N],
                            lhsT=negI[:, :],
                            rhs=mfck[:, i % MKT, n:n + MM_N],
                            start=True, stop=AB.get("no_qk", False),
                            skip_group_check=True)
                if not AB.get("no_qk"):
                    for n in range(0, QC, MM_N):
                        nc.tensor.matmul(
                            sc[:, n:n + MM_N],
                            lhsT=ktt[:, kt * P:(kt + 1) * P],
                            rhs=qt[:, qc * QC + n:qc * QC + n + MM_N],
                            start=not pe_mask, stop=True,
                            skip_group_check=True)
                # exp outputs land in [P, 2, QC] pair tiles so the acc
                # update is one FD=2048 add per k-tile pair (half the DVE
                # instructions and semaphores on the acc chain)
                if kt % 2 == 0:
                    ptp = pp.tile([P, 2, QC], fp16, name="ptpair")
                    slot = 0
                else:
                    slot = 1
                pt = ptp[:, slot, :]
                if not AB.get("no_exp"):
                    nc.scalar.activation(out=pt, in_=sc[:, :],
                                         func=Exp, scale=SCALE)
                elif const_pt is None:
                    nc.vector.tensor_copy(out=pt, in_=sc[:, :])
                do_mult = (ALL_DVE or kt % 2 != swap) \
                    and not AB.get("no_mask_mm") \
                    and not AB.get("no_mult") and not AB.get("no_exp")
                if do_mult and ALL_DVE and kt % 2 == 1:
                    # one in-place FD=2048 keep-multiply per k-tile pair
                    kbase = (kt - 1) % MKT
                    nc.vector.tensor_mul(
                        out=ptp[:, :, :], in0=ptp[:, :, :],
                        in1=kf[(kt - 1) // MKT][:, kbase:kbase + 2, :])
                elif do_mult and not ALL_DVE:
                    # in-place keep-multiply on the DVE-masked slot
                    nc.vector.tensor_mul(out=pt, in0=pt,
                                         in1=kf[i // MKT][:, i % MKT, :])
                if AB.get("no_acc") or AB.get("no_exp") or kt % 2 == 0:
                    pass
                elif kt == 1:
                    nc.vector.tensor_copy(out=acc[:, :, :],
                                          in_=ptp[:, :, :])
                else:
                    nc.vector.tensor_add(out=acc[:, :, :],
                                         in0=acc[:, :, :],
                                         in1=ptp[:, :, :])
                if const_pt is not None:
                    pts[kt] = (const_pt, 0)
                else:
                    pts[kt] = (ptp, slot)
                if not AB.get("no_pv"):
                    # ascending j so the start=True (j==0) matmul is always
                    # the first write into the ops bank
                    for j in (kt - 3, kt - 2):
                        if j < 0 or j + pv_lag(j) != kt:
                            continue
                        tj, sl = pts[j]
                        for n in range(0, QC, MM_N):
                            nc.tensor.matmul(
                                ops[:, n:n + MM_N],
                                lhsT=vsb[:, j, :],
                                rhs=tj[:, sl, n:n + MM_N],
                                start=(j == 0), stop=False,
                                skip_group_check=True)
            if not AB.get("no_pv"):
                for j in range(NKT):
                    if j + pv_lag(j) < NKT:
                        continue
                    tj, sl = pts[j]
                    for n in range(0, QC, MM_N):
                        nc.tensor.matmul(
                            ops[:, n:n + MM_N],
                            lhsT=vsb[:, j, :],
                            rhs=tj[:, sl, n:n + MM_N],
                            start=False, stop=(j == NKT - 1),
                            skip_group_check=True)

            # denominator broadcast to all partitions in one matmul:
            # den[p, q] = sum_k acc[k, q] for every p (all-ones stationary)
            den = spsum.tile([P, QC], fp32, name="scores")
            for n in range(0, QC, MM_N):
                for sl in range(2):
                    nc.tensor.matmul(den[:, n:n + MM_N],
                                     lhsT=ones_mat[:, :],
                                     rhs=acc[:, sl, n:n + MM_N],
                                     start=(sl == 0), stop=(sl == 1),
                                     skip_group_check=True)
            rcp = outp.tile([P, QC], fp32, name="rcp")
            nc.vector.reciprocal(out=rcp[:, :], in_=den[:, :])
            osf = outp.tile([P, QC], fp32, name="osf")
            nc.vector.tensor_mul(out=osf[:, :], in0=ops[:, :],
                                 in1=rcp[:, :])
            # stores go out on SWDGE: their wait (epilogue mult) parks on
            # the idle Pool engine instead of a ring the PE feed needs
            nc.gpsimd.dma_start(out=Od.ap()[b, :, qc * QC:(qc + 1) * QC],
                                in_=osf[:, :])


def _get_nc(loop=False):
    key = f"nc_loop{loop}"
    if key not in _CACHE:
        _CACHE[key] = build_nc(loop=loop)
    return _CACHE[key]


def host_prep(Q, K, V, mask):
    """Slice per core, pre-transpose Q/K/mask, pre-cast to device dtypes.

    The transposed mask is split by k-tile parity: even k-tiles are packed
    as raw fp8e4m3 bytes (0x38 = 1.0) for the PE mask-matmul path; odd
    k-tiles are packed as fp16 keep-multiplicands (1.0 = keep) for the
    DVE post-exp multiply."""
    Q = np.asarray(Q, dtype=np.float32)
    K = np.asarray(K, dtype=np.float32)
    V16 = np.asarray(V, dtype=np.float32).astype(np.float16)
    mask_u8 = np.asarray(mask).astype(np.uint8)
    Qt = np.ascontiguousarray(
        Q.transpose(0, 2, 1)).astype(np.float16)
    Kt = np.ascontiguousarray(
        K.transpose(0, 2, 1)).astype(np.float16)
    mT = mask_u8.transpose(0, 2, 1).reshape(B, NKT, P, S)
    if ALL_DVE:
        maskT = None
        keepT = np.ascontiguousarray(
            (1 - mT).astype(np.float16)).reshape(B, S, S)
    else:
        maskT = np.ascontiguousarray(
            mT[:, 1::2] * np.uint8(0x38)).reshape(B, S // 2, S)
        maskT = maskT.view(ml_dtypes.float8_e4m3)
        keepT = np.ascontiguousarray(
            (1 - mT[:, 0::2]).astype(np.float16)).reshape(B, S // 2, S)
    in_maps = []
    for c in range(NCORES):
        sl = slice(c * BP, (c + 1) * BP)
        m = {
            "Qt": np.ascontiguousarray(Qt[sl]),
            "Kt": np.ascontiguousarray(Kt[sl]),
            "V": np.ascontiguousarray(V16[sl]),
            "keepT": np.ascontiguousarray(keepT[sl]),
        }
        if maskT is not None:
            m["maskT"] = np.ascontiguousarray(maskT[sl])
        in_maps.append(m)
    return in_maps


def gather_out(results):
    """Concat per-core transposed outputs and un-transpose to [B, S, D]."""
    outT = np.concatenate([r["outT"] for r in results], axis=0)
    return np.ascontiguousarray(outT.transpose(0, 2, 1))


def kernel(Q, K, V, mask, dk=128):
    from concourse.bass_utils import run_bass_kernel_spmd

    assert int(dk) == 128
    in_maps = host_prep(Q, K, V, mask)
    nc = _get_nc(loop=False)
    res = run_bass_kernel_spmd(nc, in_maps, core_ids=list(range(NCORES)))
    return gather_out(res.results)
